# revision 21
# baseline (speedup 1.0000x reference)
"""DistMult edge scorer on 8 Trainium2 NeuronCores.

score[r, e] = sigmoid(sum_d h_u[src[r,e], d] * W[r, d] * h_v[dst[r,e], d])

Sharding: edges of each relation are sorted by source node on the host and
split into 8 contiguous slices (one per core).

Per core, per relation:
  - u side: the core's contiguous source-row range is W-prescaled on the
    host, cast to bf16, DMA'd into SBUF, and expanded per edge by bf16 PE
    one-hot matmuls (no u gathers).  Chunk t of 128 edges may only use
    source rows of the single block B_t = floor(t*NBLK/T2), fixed at
    compile time; the host greedily assigns rows to blocks (<=128 rows,
    capacity-checked) and packs edges into the blocks' chunk ranges.
  - v side: per-edge bf16 rows fetched with SWDGE dma_gather (256B rows,
    edges-on-partitions), 4 sem-congruent calls per 40-chunk batch.  The
    gather desc-gen on the Pool engine (~2.9 ns/idx) and DVE are the
    leading costs.
  - DVE builds bf16 one-hot masks via tensor_scalar is_equal (2x rate) and
    multiplies; the d-reduction is split 2/3 DVE reduce_sum / 1/3 ACT
    accumulate; ACT applies sigmoid; the host casts and unpermutes.
"""

import numpy as np
import ml_dtypes

BF16 = ml_dtypes.bfloat16

N_DRUG, N_DIS, D = 8000, 18000, 128
N_REL_DIR, E = 3, 200000
N_CORES = 8
EPC = E // N_CORES          # 25000 edges per core per relation
T2 = 208                    # chunks per (relation, core); 26624 edge slots
EL = T2 * 128

_cache = {}
_last = {}


def _blk_of(t, nb):
    return t * nb // T2


def _build_nc(cfg):
    import concourse.bacc as bacc
    import concourse.mybir as mybir
    from concourse.tile import TileContext

    f32 = mybir.dt.float32
    bf16 = mybir.dt.bfloat16
    i16 = mybir.dt.int16
    u8 = mybir.dt.uint8

    nblk_f, nblk_r, _t2 = cfg
    assert _t2 == T2
    nblk = {0: nblk_f, 1: nblk_r}

    nc = bacc.Bacc("TRN2", target_bir_lowering=False, debug=False,
                   num_devices=N_CORES, num_swdge_queues=4)

    t_hd = nc.dram_tensor("hd", (N_DRUG, D), bf16, kind="ExternalInput")
    t_hs = nc.dram_tensor("hs", (N_DIS, D), bf16, kind="ExternalInput")
    t_u = [nc.dram_tensor(f"u{r}", (nblk[r >= 3] * 128, D), bf16,
                          kind="ExternalInput") for r in range(6)]
    t_iota = nc.dram_tensor("iota", (128, 2), f32, kind="ExternalInput")
    t_ids = [nc.dram_tensor(f"ids{r}", (128, EL), bf16,
                            kind="ExternalInput") for r in range(6)]
    t_iv = [nc.dram_tensor(f"iv{r}", (128, EL // 16), i16,
                           kind="ExternalInput") for r in range(6)]
    t_out = [nc.dram_tensor(f"scores{r}", (128, T2), f32,
                            kind="ExternalOutput") for r in range(6)]

    with TileContext(nc) as tc:
        with tc.tile_pool(name="cst", bufs=1) as cst, \
             tc.tile_pool(name="mp", bufs=2) as mp, \
             tc.tile_pool(name="gp", bufs=2) as gp, \
             tc.tile_pool(name="gvp", bufs=3) as gvp, \
             tc.tile_pool(name="pp", bufs=4, space="PSUM") as pp:
            iota = cst.tile([128, 2], f32)
            nc.sync.dma_start(iota[:], t_iota[:])
            for r in range(6):
                dr = int(r >= 3)
                NB = nblk[dr]
                v_tab = t_hs if dr == 0 else t_hd

                # u range -> SBUF (row 128b+p at [p, b, :]); W-prescaled
                # bf16 on the host
                u_sb = mp.tile([128, NB, D], bf16, tag=f"usb{dr}")
                nc.sync.dma_start(
                    u_sb[:], t_u[r][:].rearrange("(b p) d -> p b d", p=128))

                iv = mp.tile([128, EL // 16], i16, tag="iv")
                nc.sync.dma_start(iv[:], t_iv[r][:])
                scores = mp.tile([128, T2], f32, tag="scores")

                batches = [40] * (T2 // 40) + ([T2 % 40] if T2 % 40 else [])
                c0 = 0
                for b, kbn in enumerate(batches):
                    nb_i = kbn * 128
                    gv = gvp.tile([128, 40, D], bf16, tag="gv")
                    # split each batch across the 4 SWDGE queues: desc-gen for
                    # queue q runs on Q7 core pair q, so the four quarters
                    # generate concurrently
                    # the queue that also carries this batch's u-gather gets
                    # a smaller v share so per-pair desc-gen is balanced
                    # fine-grained, pair-balanced issue: pairs 0/1 take
                    # 2x7 v-chunks, pairs 2/3 take 6 v-chunks (they also
                    # carry the 8-chunk u-gathers) -> 14 chunks per pair
                    gx = 0
                    qn = max(1, -(-kbn // 4))
                    segs = []
                    left, q = kbn, 0
                    while left > 0:
                        take = min(qn, left)
                        segs.append((q % 4, take))
                        left -= take
                        q += 1
                    k0 = 0
                    for q, sz in segs:
                        k1 = k0 + sz
                        nc.gpsimd.dma_gather(
                            gv[:, k0:k1, :], v_tab[:],
                            iv[:, (c0 + k0) * 8:(c0 + k1) * 8],
                            sz * 128, sz * 128, D,
                            elem_step=D, single_packet=False, queue_num=q)
                        k0 = k1
                    noh = kbn
                    ids = gp.tile([128, 40 * 128], bf16, tag="ids")
                    nc.sync.dma_start(
                        ids[:, :noh * 128],
                        t_ids[r][:, c0 * 128:(c0 + kbn) * 128])
                    oh_lo = gp.tile([128, 40 * 128], bf16, tag="ohlo")
                    nc.vector.tensor_scalar(
                        oh_lo[:, :noh * 128], ids[:, :noh * 128],
                        iota[:, 1:2], None, op0=mybir.AluOpType.is_equal)
                    for g0 in range(0, kbn, 4):
                        gn = min(4, kbn - g0)
                        if g0 + gn <= gx:
                            usrc = gu[:, g0:g0 + gn, :]
                        elif g0 >= gx:
                            ps = pp.tile([128, 4, D], f32, tag="ps")
                            for i in range(g0, g0 + gn):
                                t = c0 + i
                                blk = _blk_of(t, NB)
                                j = i - gx
                                nc.tensor.matmul(
                                    ps[:, i - g0, :],
                                    lhsT=oh_lo[:, j * 128:(j + 1) * 128],
                                    rhs=u_sb[:, blk, :],
                                    start=True, stop=True)
                            usrc = ps[:, :gn, :]
                        else:
                            raise AssertionError("gx must be multiple of 4")
                        prod = gp.tile([128, 4, D], f32, tag="prod")
                        nc.vector.tensor_tensor(
                            prod[:, :gn, :].rearrange("p a b -> p (a b)"),
                            usrc.rearrange("p a b -> p (a b)"),
                            gv[:, g0:g0 + gn, :].rearrange("p a b -> p (a b)"),
                            op=mybir.AluOpType.mult)
                        # reduction split between scalar engine (4x slower
                        # per chunk but otherwise idle) and DVE
                        if (g0 // 4) % 3 != 0:
                            nc.vector.reduce_sum(
                                scores[:, c0 + g0:c0 + g0 + gn],
                                prod[:, :gn, :], axis=mybir.AxisListType.X)
                        else:
                            acts = cst.tile([128, D], f32, tag="actout")
                            for i in range(gn):
                                nc.scalar.activation(
                                    acts[:], prod[:, i, :],
                                    mybir.ActivationFunctionType.Copy,
                                    accum_out=scores[:, c0 + g0 + i:c0 + g0 + i + 1])
                    c0 += kbn

                sig = mp.tile([128, T2], f32, tag="sig")
                nc.scalar.activation(
                    sig[:], scores[:], mybir.ActivationFunctionType.Sigmoid)
                nc.sync.dma_start(t_out[r][:], sig[:])

    nc.compile()
    return nc


def _wrap_idx(idx):
    n = idx.shape[0]
    w = idx.reshape(n // 16, 16).T.astype(np.int16)
    return np.ascontiguousarray(np.tile(w, (8, 1)))


def _pack_schedule(u_local, v_idx, nblk):
    """Pack edges (sorted by u_local) into T2 chunks of 128 where chunk t may
    only use rows assigned to block blk_t = t*nblk//T2, at most 128 distinct
    rows per block.  Returns (ids, v16, edge_of_slot, vpos) or None."""
    rows, starts, counts = np.unique(u_local, return_index=True,
                                     return_counts=True)
    nrows = rows.shape[0]
    ids = np.zeros(EL, np.uint8)
    v16 = np.zeros(EL, np.int16)
    eos = np.full(EL, -1, np.int64)
    vpos = np.full(int(u_local[-1]) + 1, -1, np.int64)

    blk_of_t = np.array([_blk_of(t, nblk) for t in range(T2)], np.int64)
    t_first = np.searchsorted(blk_of_t, np.arange(nblk), side="left")
    t_last = np.searchsorted(blk_of_t, np.arange(nblk), side="right")
    ri = 0
    for b in range(nblk):
        cap = 128 * int(t_last[b] - t_first[b])
        slot0 = 128 * int(t_first[b])
        used = 0
        rib = 0
        while ri < nrows and rib < 128 and used + int(counts[ri]) <= cap:
            c = int(counts[ri])
            sx = int(starts[ri])
            sl = slot0 + used
            ids[sl:sl + c] = rib
            v16[sl:sl + c] = v_idx[sx:sx + c].astype(np.int16)
            eos[sl:sl + c] = np.arange(sx, sx + c)
            vpos[int(rows[ri])] = 128 * b + rib
            used += c
            rib += 1
            ri += 1
    if ri != nrows:
        return None
    return ids, v16, eos, vpos


def _prepare(rels, sliced, nblk_f, nblk_r, W, iota):
    slot_maps = [[None] * N_CORES for _ in range(6)]
    in_maps = []
    hd16 = rels[0][2].astype(BF16)
    hs16 = rels[3][2].astype(BF16)
    for c in range(N_CORES):
        m = {"hd": hd16, "hs": hs16, "iota": iota}
        for r in range(6):
            dr = int(r >= 3)
            nblk = nblk_f if dr == 0 else nblk_r
            u_local, v_idx, lo = sliced[r][c]
            packed = _pack_schedule(u_local, v_idx, nblk)
            if packed is None:
                return None, None, (r, c)
            ids, v16, eos, vpos = packed
            tab = rels[r][2]
            span = vpos.shape[0]
            urows = np.zeros((nblk * 128, D), np.float32)
            valid = vpos >= 0
            urows[vpos[valid]] = (tab[lo:lo + span][valid]
                                  * W[r][None, :]).astype(np.float32)
            m[f"u{r}"] = urows.astype(BF16)
            m[f"ids{r}"] = np.ascontiguousarray(
                np.broadcast_to(ids.astype(BF16)[None, :], (128, EL)))
            m[f"iv{r}"] = _wrap_idx(v16)
            slot_maps[r][c] = eos
        in_maps.append(m)
    return slot_maps, in_maps, None


def kernel(h_drug, h_disease, W, drug_src, dis_dst, dis_src, drug_dst):
    from concourse.bass_utils import run_bass_kernel_spmd

    h_drug = np.asarray(h_drug, dtype=np.float32)
    h_disease = np.asarray(h_disease, dtype=np.float32)
    W = np.asarray(W, dtype=np.float32)

    rels = []
    for r in range(3):
        rels.append((np.asarray(drug_src[r]), np.asarray(dis_dst[r]), h_drug))
    for r in range(3):
        rels.append((np.asarray(dis_src[r]), np.asarray(drug_dst[r]), h_disease))

    perms = []
    sliced = []
    for r in range(6):
        u_idx, v_idx, _ = rels[r]
        perm = np.argsort(u_idx, kind="stable")
        perms.append(perm)
        us, vs = u_idx[perm], v_idx[perm]
        sl = []
        for c in range(N_CORES):
            ui = us[c * EPC:(c + 1) * EPC]
            vi = vs[c * EPC:(c + 1) * EPC]
            lo = int(ui[0])
            sl.append((ui - lo, vi, lo))
        sliced.append(sl)

    def span_max(dr):
        sp = 0
        for r in (range(3) if dr == 0 else range(3, 6)):
            for c in range(N_CORES):
                sp = max(sp, int(sliced[r][c][0][-1]) + 1)
        return sp

    nblk_f = max(2, -(-span_max(0) // 112))
    nblk_r = max(2, -(-span_max(1) // 112))

    iota = np.empty((128, 2), np.float32)
    iota[:, 0] = np.arange(128)
    iota[:, 1] = np.arange(128)

    slot_maps = in_maps = None
    for _attempt in range(6):
        slot_maps, in_maps, fail = _prepare(rels, sliced, nblk_f, nblk_r,
                                            W, iota)
        if fail is None:
            break
        if fail[0] < 3:
            nblk_f += 1
        else:
            nblk_r += 1
    else:
        raise RuntimeError("could not build a feasible chunk schedule")

    cfg = (nblk_f, nblk_r, T2)
    if cfg not in _cache:
        _cache[cfg] = _build_nc(cfg)
    nc = _cache[cfg]

    res = run_bass_kernel_spmd(nc, in_maps, core_ids=list(range(N_CORES)))
    _last["exec_time_ns"] = res.exec_time_ns
    if res.instructions_and_trace is not None:
        _last["trace_path"] = res.instructions_and_trace[1]

    out = np.empty((6, E), np.float32)
    for r in range(6):
        sorted_scores = np.empty(EPC * N_CORES, np.float32)
        for c in range(N_CORES):
            s = res.results[c][f"scores{r}"]       # [128, T2]
            flat = s.T.reshape(-1)                 # slot j = t*128+p
            eos = slot_maps[r][c]
            valid = eos >= 0
            sorted_scores[c * EPC + eos[valid]] = flat[valid]
        out[r, perms[r]] = sorted_scores
    return out



# revision 22
# speedup vs baseline: 1.0146x; 1.0146x over previous
"""DistMult edge scorer on 8 Trainium2 NeuronCores.

score[r, e] = sigmoid(sum_d h_u[src[r,e], d] * W[r, d] * h_v[dst[r,e], d])

Sharding: edges of each relation are sorted by source node on the host and
split into 8 contiguous slices (one per core).

Per core, per relation:
  - u side: the core's contiguous source-row range is W-prescaled on the
    host, cast to bf16, DMA'd into SBUF, and expanded per edge by bf16 PE
    one-hot matmuls (no u gathers).  Chunk t of 128 edges may only use
    source rows of the single block B_t = floor(t*NBLK/T2), fixed at
    compile time; the host greedily assigns rows to blocks (<=128 rows,
    capacity-checked) and packs edges into the blocks' chunk ranges.
  - v side: per-edge bf16 rows fetched with SWDGE dma_gather (256B rows,
    edges-on-partitions), 4 sem-congruent calls per 40-chunk batch.  The
    gather desc-gen on the Pool engine (~2.9 ns/idx) and DVE are the
    leading costs.
  - DVE builds bf16 one-hot masks via tensor_scalar is_equal (2x rate) and
    multiplies; the d-reduction is split 2/3 DVE reduce_sum / 1/3 ACT
    accumulate; ACT applies sigmoid; the host casts and unpermutes.
"""

import numpy as np
import ml_dtypes

BF16 = ml_dtypes.bfloat16

N_DRUG, N_DIS, D = 8000, 18000, 128
N_REL_DIR, E = 3, 200000
N_CORES = 8
EPC = E // N_CORES          # 25000 edges per core per relation
T2 = 208                    # chunks per (relation, core); 26624 edge slots
EL = T2 * 128

_cache = {}
_last = {}


def _blk_of(t, nb):
    return t * nb // T2


def _build_nc(cfg):
    import concourse.bacc as bacc
    import concourse.mybir as mybir
    from concourse.tile import TileContext

    f32 = mybir.dt.float32
    bf16 = mybir.dt.bfloat16
    i16 = mybir.dt.int16
    u8 = mybir.dt.uint8

    nblk_f, nblk_r, _t2 = cfg
    assert _t2 == T2
    nblk = {0: nblk_f, 1: nblk_r}

    nc = bacc.Bacc("TRN2", target_bir_lowering=False, debug=False,
                   num_devices=N_CORES, num_swdge_queues=4)

    t_hd = nc.dram_tensor("hd", (N_DRUG, D), bf16, kind="ExternalInput")
    t_hs = nc.dram_tensor("hs", (N_DIS, D), bf16, kind="ExternalInput")
    t_u = [nc.dram_tensor(f"u{r}", (nblk[r >= 3] * 128, D), bf16,
                          kind="ExternalInput") for r in range(6)]
    t_iota = nc.dram_tensor("iota", (128, 2), f32, kind="ExternalInput")
    t_ids = [nc.dram_tensor(f"ids{r}", (128, EL), u8,
                            kind="ExternalInput") for r in range(6)]
    t_iv = [nc.dram_tensor(f"iv{r}", (128, EL // 16), i16,
                           kind="ExternalInput") for r in range(6)]
    t_out = [nc.dram_tensor(f"scores{r}", (128, T2), f32,
                            kind="ExternalOutput") for r in range(6)]

    with TileContext(nc) as tc:
        with tc.tile_pool(name="cst", bufs=1) as cst, \
             tc.tile_pool(name="mp", bufs=2) as mp, \
             tc.tile_pool(name="gp", bufs=2) as gp, \
             tc.tile_pool(name="gvp", bufs=3) as gvp, \
             tc.tile_pool(name="pp", bufs=4, space="PSUM") as pp:
            iota = cst.tile([128, 2], f32)
            nc.sync.dma_start(iota[:], t_iota[:])
            for r in range(6):
                dr = int(r >= 3)
                NB = nblk[dr]
                v_tab = t_hs if dr == 0 else t_hd

                # u range -> SBUF (row 128b+p at [p, b, :]); W-prescaled
                # bf16 on the host
                u_sb = mp.tile([128, NB, D], bf16, tag=f"usb{dr}")
                nc.sync.dma_start(
                    u_sb[:], t_u[r][:].rearrange("(b p) d -> p b d", p=128))

                iv = mp.tile([128, EL // 16], i16, tag="iv")
                nc.sync.dma_start(iv[:], t_iv[r][:])
                scores = mp.tile([128, T2], f32, tag="scores")

                batches = [40] * (T2 // 40) + ([T2 % 40] if T2 % 40 else [])
                c0 = 0
                for b, kbn in enumerate(batches):
                    nb_i = kbn * 128
                    gv = gvp.tile([128, 40, D], bf16, tag="gv")
                    # split each batch across the 4 SWDGE queues: desc-gen for
                    # queue q runs on Q7 core pair q, so the four quarters
                    # generate concurrently
                    # the queue that also carries this batch's u-gather gets
                    # a smaller v share so per-pair desc-gen is balanced
                    # fine-grained, pair-balanced issue: pairs 0/1 take
                    # 2x7 v-chunks, pairs 2/3 take 6 v-chunks (they also
                    # carry the 8-chunk u-gathers) -> 14 chunks per pair
                    gx = 0
                    qn = max(1, -(-kbn // 4))
                    segs = []
                    left, q = kbn, 0
                    while left > 0:
                        take = min(qn, left)
                        segs.append((q % 4, take))
                        left -= take
                        q += 1
                    k0 = 0
                    for q, sz in segs:
                        k1 = k0 + sz
                        nc.gpsimd.dma_gather(
                            gv[:, k0:k1, :], v_tab[:],
                            iv[:, (c0 + k0) * 8:(c0 + k1) * 8],
                            sz * 128, sz * 128, D,
                            elem_step=D, single_packet=False, queue_num=q)
                        k0 = k1
                    noh = kbn
                    ids = gp.tile([128, 40 * 128], u8, tag="ids")
                    nc.sync.dma_start(
                        ids[:, :noh * 128],
                        t_ids[r][:, c0 * 128:(c0 + kbn) * 128])
                    oh_lo = gp.tile([128, 40 * 128], bf16, tag="ohlo")
                    nc.vector.tensor_scalar(
                        oh_lo[:, :noh * 128], ids[:, :noh * 128],
                        iota[:, 1:2], None, op0=mybir.AluOpType.is_equal)
                    for g0 in range(0, kbn, 4):
                        gn = min(4, kbn - g0)
                        if g0 + gn <= gx:
                            usrc = gu[:, g0:g0 + gn, :]
                        elif g0 >= gx:
                            ps = pp.tile([128, 4, D], f32, tag="ps")
                            for i in range(g0, g0 + gn):
                                t = c0 + i
                                blk = _blk_of(t, NB)
                                j = i - gx
                                nc.tensor.matmul(
                                    ps[:, i - g0, :],
                                    lhsT=oh_lo[:, j * 128:(j + 1) * 128],
                                    rhs=u_sb[:, blk, :],
                                    start=True, stop=True)
                            usrc = ps[:, :gn, :]
                        else:
                            raise AssertionError("gx must be multiple of 4")
                        prod = gp.tile([128, 4, D], f32, tag="prod")
                        nc.vector.tensor_tensor(
                            prod[:, :gn, :].rearrange("p a b -> p (a b)"),
                            usrc.rearrange("p a b -> p (a b)"),
                            gv[:, g0:g0 + gn, :].rearrange("p a b -> p (a b)"),
                            op=mybir.AluOpType.mult)
                        # reduction split between scalar engine (4x slower
                        # per chunk but otherwise idle) and DVE
                        if (g0 // 4) % 3 != 0:
                            nc.vector.reduce_sum(
                                scores[:, c0 + g0:c0 + g0 + gn],
                                prod[:, :gn, :], axis=mybir.AxisListType.X)
                        else:
                            acts = cst.tile([128, D], f32, tag="actout")
                            for i in range(gn):
                                nc.scalar.activation(
                                    acts[:], prod[:, i, :],
                                    mybir.ActivationFunctionType.Copy,
                                    accum_out=scores[:, c0 + g0 + i:c0 + g0 + i + 1])
                    c0 += kbn

                sig = mp.tile([128, T2], f32, tag="sig")
                nc.scalar.activation(
                    sig[:], scores[:], mybir.ActivationFunctionType.Sigmoid)
                nc.sync.dma_start(t_out[r][:], sig[:])

    nc.compile()
    return nc


def _wrap_idx(idx):
    n = idx.shape[0]
    w = idx.reshape(n // 16, 16).T.astype(np.int16)
    return np.ascontiguousarray(np.tile(w, (8, 1)))


def _pack_schedule(u_local, v_idx, nblk):
    """Pack edges (sorted by u_local) into T2 chunks of 128 where chunk t may
    only use rows assigned to block blk_t = t*nblk//T2, at most 128 distinct
    rows per block.  Returns (ids, v16, edge_of_slot, vpos) or None."""
    rows, starts, counts = np.unique(u_local, return_index=True,
                                     return_counts=True)
    nrows = rows.shape[0]
    ids = np.zeros(EL, np.uint8)
    v16 = np.zeros(EL, np.int16)
    eos = np.full(EL, -1, np.int64)
    vpos = np.full(int(u_local[-1]) + 1, -1, np.int64)

    blk_of_t = np.array([_blk_of(t, nblk) for t in range(T2)], np.int64)
    t_first = np.searchsorted(blk_of_t, np.arange(nblk), side="left")
    t_last = np.searchsorted(blk_of_t, np.arange(nblk), side="right")
    ri = 0
    for b in range(nblk):
        cap = 128 * int(t_last[b] - t_first[b])
        slot0 = 128 * int(t_first[b])
        used = 0
        rib = 0
        while ri < nrows and rib < 128 and used + int(counts[ri]) <= cap:
            c = int(counts[ri])
            sx = int(starts[ri])
            sl = slot0 + used
            ids[sl:sl + c] = rib
            v16[sl:sl + c] = v_idx[sx:sx + c].astype(np.int16)
            eos[sl:sl + c] = np.arange(sx, sx + c)
            vpos[int(rows[ri])] = 128 * b + rib
            used += c
            rib += 1
            ri += 1
    if ri != nrows:
        return None
    return ids, v16, eos, vpos


def _prepare(rels, sliced, nblk_f, nblk_r, W, iota):
    slot_maps = [[None] * N_CORES for _ in range(6)]
    in_maps = []
    hd16 = rels[0][2].astype(BF16)
    hs16 = rels[3][2].astype(BF16)
    for c in range(N_CORES):
        m = {"hd": hd16, "hs": hs16, "iota": iota}
        for r in range(6):
            dr = int(r >= 3)
            nblk = nblk_f if dr == 0 else nblk_r
            u_local, v_idx, lo = sliced[r][c]
            packed = _pack_schedule(u_local, v_idx, nblk)
            if packed is None:
                return None, None, (r, c)
            ids, v16, eos, vpos = packed
            tab = rels[r][2]
            span = vpos.shape[0]
            urows = np.zeros((nblk * 128, D), np.float32)
            valid = vpos >= 0
            urows[vpos[valid]] = (tab[lo:lo + span][valid]
                                  * W[r][None, :]).astype(np.float32)
            m[f"u{r}"] = urows.astype(BF16)
            m[f"ids{r}"] = np.ascontiguousarray(
                np.broadcast_to(ids[None, :], (128, EL)))
            m[f"iv{r}"] = _wrap_idx(v16)
            slot_maps[r][c] = eos
        in_maps.append(m)
    return slot_maps, in_maps, None


def kernel(h_drug, h_disease, W, drug_src, dis_dst, dis_src, drug_dst):
    from concourse.bass_utils import run_bass_kernel_spmd

    h_drug = np.asarray(h_drug, dtype=np.float32)
    h_disease = np.asarray(h_disease, dtype=np.float32)
    W = np.asarray(W, dtype=np.float32)

    rels = []
    for r in range(3):
        rels.append((np.asarray(drug_src[r]), np.asarray(dis_dst[r]), h_drug))
    for r in range(3):
        rels.append((np.asarray(dis_src[r]), np.asarray(drug_dst[r]), h_disease))

    perms = []
    sliced = []
    for r in range(6):
        u_idx, v_idx, _ = rels[r]
        perm = np.argsort(u_idx, kind="stable")
        perms.append(perm)
        us, vs = u_idx[perm], v_idx[perm]
        sl = []
        for c in range(N_CORES):
            ui = us[c * EPC:(c + 1) * EPC]
            vi = vs[c * EPC:(c + 1) * EPC]
            lo = int(ui[0])
            sl.append((ui - lo, vi, lo))
        sliced.append(sl)

    def span_max(dr):
        sp = 0
        for r in (range(3) if dr == 0 else range(3, 6)):
            for c in range(N_CORES):
                sp = max(sp, int(sliced[r][c][0][-1]) + 1)
        return sp

    nblk_f = max(2, -(-span_max(0) // 112))
    nblk_r = max(2, -(-span_max(1) // 112))

    iota = np.empty((128, 2), np.float32)
    iota[:, 0] = np.arange(128)
    iota[:, 1] = np.arange(128)

    slot_maps = in_maps = None
    for _attempt in range(6):
        slot_maps, in_maps, fail = _prepare(rels, sliced, nblk_f, nblk_r,
                                            W, iota)
        if fail is None:
            break
        if fail[0] < 3:
            nblk_f += 1
        else:
            nblk_r += 1
    else:
        raise RuntimeError("could not build a feasible chunk schedule")

    cfg = (nblk_f, nblk_r, T2)
    if cfg not in _cache:
        _cache[cfg] = _build_nc(cfg)
    nc = _cache[cfg]

    res = run_bass_kernel_spmd(nc, in_maps, core_ids=list(range(N_CORES)))
    _last["exec_time_ns"] = res.exec_time_ns
    if res.instructions_and_trace is not None:
        _last["trace_path"] = res.instructions_and_trace[1]

    out = np.empty((6, E), np.float32)
    for r in range(6):
        sorted_scores = np.empty(EPC * N_CORES, np.float32)
        for c in range(N_CORES):
            s = res.results[c][f"scores{r}"]       # [128, T2]
            flat = s.T.reshape(-1)                 # slot j = t*128+p
            eos = slot_maps[r][c]
            valid = eos >= 0
            sorted_scores[c * EPC + eos[valid]] = flat[valid]
        out[r, perms[r]] = sorted_scores
    return out



# revision 23
# speedup vs baseline: 1.1138x; 1.0977x over previous
"""DistMult edge scorer on 8 Trainium2 NeuronCores.

score[r, e] = sigmoid(sum_d h_u[src[r,e], d] * W[r, d] * h_v[dst[r,e], d])

Sharding: edges of each relation are sorted by source node on the host and
split into 8 contiguous slices (one per core).

Per core, per relation:
  - u side: the core's contiguous source-row range is W-prescaled on the
    host, cast to bf16, DMA'd into SBUF, and expanded per edge by bf16 PE
    one-hot matmuls (no u gathers).  Chunk t of 128 edges may only use
    source rows of the single block B_t = floor(t*NBLK/T2), fixed at
    compile time; the host greedily assigns rows to blocks (<=128 rows,
    capacity-checked) and packs edges into the blocks' chunk ranges.
  - v side: per-edge bf16 rows fetched with SWDGE dma_gather (256B rows,
    edges-on-partitions), 4 sem-congruent calls per 40-chunk batch.  The
    gather desc-gen on the Pool engine (~2.9 ns/idx) and DVE are the
    leading costs.
  - DVE builds bf16 one-hot masks via tensor_scalar is_equal (2x rate) and
    multiplies; the d-reduction is split 2/3 DVE reduce_sum / 1/3 ACT
    accumulate; ACT applies sigmoid; the host casts and unpermutes.
"""

import numpy as np
import ml_dtypes

BF16 = ml_dtypes.bfloat16

N_DRUG, N_DIS, D = 8000, 18000, 128
N_REL_DIR, E = 3, 200000
N_CORES = 8
EPC = E // N_CORES          # 25000 edges per core per relation
T2 = 200                    # chunks per (relation, core); 25600 edge slots
EL = T2 * 128

_cache = {}
_last = {}


def _blk_of(t, nb):
    return t * nb // T2


def _build_nc(cfg):
    import concourse.bacc as bacc
    import concourse.mybir as mybir
    from concourse.tile import TileContext

    f32 = mybir.dt.float32
    bf16 = mybir.dt.bfloat16
    i16 = mybir.dt.int16
    u8 = mybir.dt.uint8

    nblk_f, nblk_r, _t2 = cfg
    assert _t2 == T2
    nblk = {0: nblk_f, 1: nblk_r}

    nc = bacc.Bacc("TRN2", target_bir_lowering=False, debug=False,
                   num_devices=N_CORES, num_swdge_queues=4)

    t_hd = nc.dram_tensor("hd", (N_DRUG, D), bf16, kind="ExternalInput")
    t_hs = nc.dram_tensor("hs", (N_DIS, D), bf16, kind="ExternalInput")
    t_u = [nc.dram_tensor(f"u{r}", (nblk[r >= 3] * 128, D), bf16,
                          kind="ExternalInput") for r in range(6)]
    t_iota = nc.dram_tensor("iota", (128, 2), f32, kind="ExternalInput")
    t_ids = [nc.dram_tensor(f"ids{r}", (128, EL), u8,
                            kind="ExternalInput") for r in range(6)]
    t_iv = [nc.dram_tensor(f"iv{r}", (128, EL // 16), i16,
                           kind="ExternalInput") for r in range(6)]
    t_out = [nc.dram_tensor(f"scores{r}", (128, T2), f32,
                            kind="ExternalOutput") for r in range(6)]

    with TileContext(nc) as tc:
        with tc.tile_pool(name="cst", bufs=1) as cst, \
             tc.tile_pool(name="mp", bufs=2) as mp, \
             tc.tile_pool(name="gp", bufs=2) as gp, \
             tc.tile_pool(name="gvp", bufs=4) as gvp, \
             tc.tile_pool(name="pp", bufs=4, space="PSUM") as pp:
            iota = cst.tile([128, 2], f32)
            nc.sync.dma_start(iota[:], t_iota[:])
            for r in range(6):
                dr = int(r >= 3)
                NB = nblk[dr]
                v_tab = t_hs if dr == 0 else t_hd

                # u range -> SBUF (row 128b+p at [p, b, :]); W-prescaled
                # bf16 on the host
                u_sb = mp.tile([128, NB, D], bf16, tag=f"usb{dr}")
                nc.sync.dma_start(
                    u_sb[:], t_u[r][:].rearrange("(b p) d -> p b d", p=128))

                iv = mp.tile([128, EL // 16], i16, tag="iv")
                nc.sync.dma_start(iv[:], t_iv[r][:])
                scores = mp.tile([128, T2], f32, tag="scores")

                batches = [40] * (T2 // 40) + ([T2 % 40] if T2 % 40 else [])
                c0 = 0
                for b, kbn in enumerate(batches):
                    nb_i = kbn * 128
                    gv = gvp.tile([128, 40, D], bf16, tag="gv")
                    # split each batch across the 4 SWDGE queues: desc-gen for
                    # queue q runs on Q7 core pair q, so the four quarters
                    # generate concurrently
                    # the queue that also carries this batch's u-gather gets
                    # a smaller v share so per-pair desc-gen is balanced
                    # fine-grained, pair-balanced issue: pairs 0/1 take
                    # 2x7 v-chunks, pairs 2/3 take 6 v-chunks (they also
                    # carry the 8-chunk u-gathers) -> 14 chunks per pair
                    gx = 0
                    qn = max(1, -(-kbn // 4))
                    segs = []
                    left, q = kbn, 0
                    while left > 0:
                        take = min(qn, left)
                        segs.append((q % 4, take))
                        left -= take
                        q += 1
                    k0 = 0
                    for q, sz in segs:
                        k1 = k0 + sz
                        nc.gpsimd.dma_gather(
                            gv[:, k0:k1, :], v_tab[:],
                            iv[:, (c0 + k0) * 8:(c0 + k1) * 8],
                            sz * 128, sz * 128, D,
                            elem_step=D, single_packet=False, queue_num=q)
                        k0 = k1
                    noh = kbn
                    ids = gp.tile([128, 40 * 128], u8, tag="ids")
                    nc.sync.dma_start(
                        ids[:, :noh * 128],
                        t_ids[r][:, c0 * 128:(c0 + kbn) * 128])
                    oh_lo = gp.tile([128, 40 * 128], bf16, tag="ohlo")
                    nc.vector.tensor_scalar(
                        oh_lo[:, :noh * 128], ids[:, :noh * 128],
                        iota[:, 1:2], None, op0=mybir.AluOpType.is_equal)
                    for g0 in range(0, kbn, 4):
                        gn = min(4, kbn - g0)
                        if g0 + gn <= gx:
                            usrc = gu[:, g0:g0 + gn, :]
                        elif g0 >= gx:
                            ps = pp.tile([128, 4, D], f32, tag="ps")
                            for i in range(g0, g0 + gn):
                                t = c0 + i
                                blk = _blk_of(t, NB)
                                j = i - gx
                                nc.tensor.matmul(
                                    ps[:, i - g0, :],
                                    lhsT=oh_lo[:, j * 128:(j + 1) * 128],
                                    rhs=u_sb[:, blk, :],
                                    start=True, stop=True)
                            usrc = ps[:, :gn, :]
                        else:
                            raise AssertionError("gx must be multiple of 4")
                        prod = gp.tile([128, 4, D], f32, tag="prod")
                        nc.vector.tensor_tensor(
                            prod[:, :gn, :].rearrange("p a b -> p (a b)"),
                            usrc.rearrange("p a b -> p (a b)"),
                            gv[:, g0:g0 + gn, :].rearrange("p a b -> p (a b)"),
                            op=mybir.AluOpType.mult)
                        # reduction split between scalar engine (4x slower
                        # per chunk but otherwise idle) and DVE
                        if (g0 // 4) % 3 != 0:
                            nc.vector.reduce_sum(
                                scores[:, c0 + g0:c0 + g0 + gn],
                                prod[:, :gn, :], axis=mybir.AxisListType.X)
                        else:
                            acts = cst.tile([128, D], f32, tag="actout")
                            for i in range(gn):
                                nc.scalar.activation(
                                    acts[:], prod[:, i, :],
                                    mybir.ActivationFunctionType.Copy,
                                    accum_out=scores[:, c0 + g0 + i:c0 + g0 + i + 1])
                    c0 += kbn

                sig = mp.tile([128, T2], f32, tag="sig")
                nc.scalar.activation(
                    sig[:], scores[:], mybir.ActivationFunctionType.Sigmoid)
                nc.sync.dma_start(t_out[r][:], sig[:])

    nc.compile()
    return nc


def _wrap_idx(idx):
    n = idx.shape[0]
    w = idx.reshape(n // 16, 16).T.astype(np.int16)
    return np.ascontiguousarray(np.tile(w, (8, 1)))


def _pack_schedule(u_local, v_idx, nblk):
    """Pack edges (sorted by u_local) into T2 chunks of 128 where chunk t may
    only use rows assigned to block blk_t = t*nblk//T2, at most 128 distinct
    rows per block.  Returns (ids, v16, edge_of_slot, vpos) or None."""
    rows, starts, counts = np.unique(u_local, return_index=True,
                                     return_counts=True)
    nrows = rows.shape[0]
    ids = np.zeros(EL, np.uint8)
    v16 = np.zeros(EL, np.int16)
    eos = np.full(EL, -1, np.int64)
    vpos = np.full(int(u_local[-1]) + 1, -1, np.int64)

    blk_of_t = np.array([_blk_of(t, nblk) for t in range(T2)], np.int64)
    t_first = np.searchsorted(blk_of_t, np.arange(nblk), side="left")
    t_last = np.searchsorted(blk_of_t, np.arange(nblk), side="right")
    ri = 0
    for b in range(nblk):
        cap = 128 * int(t_last[b] - t_first[b])
        slot0 = 128 * int(t_first[b])
        used = 0
        rib = 0
        while ri < nrows and rib < 128 and used + int(counts[ri]) <= cap:
            c = int(counts[ri])
            sx = int(starts[ri])
            sl = slot0 + used
            ids[sl:sl + c] = rib
            v16[sl:sl + c] = v_idx[sx:sx + c].astype(np.int16)
            eos[sl:sl + c] = np.arange(sx, sx + c)
            vpos[int(rows[ri])] = 128 * b + rib
            used += c
            rib += 1
            ri += 1
    if ri != nrows:
        return None
    return ids, v16, eos, vpos


def _prepare(rels, sliced, nblk_f, nblk_r, W, iota):
    slot_maps = [[None] * N_CORES for _ in range(6)]
    in_maps = []
    hd16 = rels[0][2].astype(BF16)
    hs16 = rels[3][2].astype(BF16)
    for c in range(N_CORES):
        m = {"hd": hd16, "hs": hs16, "iota": iota}
        for r in range(6):
            dr = int(r >= 3)
            nblk = nblk_f if dr == 0 else nblk_r
            u_local, v_idx, lo = sliced[r][c]
            packed = _pack_schedule(u_local, v_idx, nblk)
            if packed is None:
                return None, None, (r, c)
            ids, v16, eos, vpos = packed
            tab = rels[r][2]
            span = vpos.shape[0]
            urows = np.zeros((nblk * 128, D), np.float32)
            valid = vpos >= 0
            urows[vpos[valid]] = (tab[lo:lo + span][valid]
                                  * W[r][None, :]).astype(np.float32)
            m[f"u{r}"] = urows.astype(BF16)
            m[f"ids{r}"] = np.ascontiguousarray(
                np.broadcast_to(ids[None, :], (128, EL)))
            m[f"iv{r}"] = _wrap_idx(v16)
            slot_maps[r][c] = eos
        in_maps.append(m)
    return slot_maps, in_maps, None


def kernel(h_drug, h_disease, W, drug_src, dis_dst, dis_src, drug_dst):
    from concourse.bass_utils import run_bass_kernel_spmd

    h_drug = np.asarray(h_drug, dtype=np.float32)
    h_disease = np.asarray(h_disease, dtype=np.float32)
    W = np.asarray(W, dtype=np.float32)

    rels = []
    for r in range(3):
        rels.append((np.asarray(drug_src[r]), np.asarray(dis_dst[r]), h_drug))
    for r in range(3):
        rels.append((np.asarray(dis_src[r]), np.asarray(drug_dst[r]), h_disease))

    perms = []
    sliced = []
    for r in range(6):
        u_idx, v_idx, _ = rels[r]
        perm = np.argsort(u_idx, kind="stable")
        perms.append(perm)
        us, vs = u_idx[perm], v_idx[perm]
        sl = []
        for c in range(N_CORES):
            ui = us[c * EPC:(c + 1) * EPC]
            vi = vs[c * EPC:(c + 1) * EPC]
            lo = int(ui[0])
            sl.append((ui - lo, vi, lo))
        sliced.append(sl)

    def span_max(dr):
        sp = 0
        for r in (range(3) if dr == 0 else range(3, 6)):
            for c in range(N_CORES):
                sp = max(sp, int(sliced[r][c][0][-1]) + 1)
        return sp

    nblk_f = max(2, -(-span_max(0) // 112))
    nblk_r = max(2, -(-span_max(1) // 112))

    iota = np.empty((128, 2), np.float32)
    iota[:, 0] = np.arange(128)
    iota[:, 1] = np.arange(128)

    slot_maps = in_maps = None
    for _attempt in range(6):
        slot_maps, in_maps, fail = _prepare(rels, sliced, nblk_f, nblk_r,
                                            W, iota)
        if fail is None:
            break
        if fail[0] < 3:
            nblk_f += 1
        else:
            nblk_r += 1
    else:
        raise RuntimeError("could not build a feasible chunk schedule")

    cfg = (nblk_f, nblk_r, T2)
    if cfg not in _cache:
        _cache[cfg] = _build_nc(cfg)
    nc = _cache[cfg]

    res = run_bass_kernel_spmd(nc, in_maps, core_ids=list(range(N_CORES)))
    _last["exec_time_ns"] = res.exec_time_ns
    if res.instructions_and_trace is not None:
        _last["trace_path"] = res.instructions_and_trace[1]

    out = np.empty((6, E), np.float32)
    for r in range(6):
        sorted_scores = np.empty(EPC * N_CORES, np.float32)
        for c in range(N_CORES):
            s = res.results[c][f"scores{r}"]       # [128, T2]
            flat = s.T.reshape(-1)                 # slot j = t*128+p
            eos = slot_maps[r][c]
            valid = eos >= 0
            sorted_scores[c * EPC + eos[valid]] = flat[valid]
        out[r, perms[r]] = sorted_scores
    return out



# revision 24
# speedup vs baseline: 1.2087x; 1.0852x over previous
"""DistMult edge scorer on 8 Trainium2 NeuronCores.

score[r, e] = sigmoid(sum_d h_u[src[r,e], d] * W[r, d] * h_v[dst[r,e], d])

Sharding: edges of each relation are sorted by source node on the host and
split into 8 contiguous slices (one per core).

Per core, per relation:
  - u side: the core's contiguous source-row range is W-prescaled on the
    host, cast to bf16, DMA'd into SBUF, and expanded per edge by bf16 PE
    one-hot matmuls (no u gathers).  Chunk t of 128 edges may only use
    source rows of the single block B_t = floor(t*NBLK/T2), fixed at
    compile time; the host greedily assigns rows to blocks (<=128 rows,
    capacity-checked) and packs edges into the blocks' chunk ranges.
  - v side: per-edge bf16 rows fetched with SWDGE dma_gather (256B rows,
    edges-on-partitions), 4 sem-congruent calls per 40-chunk batch.  The
    gather desc-gen on the Pool engine (~2.9 ns/idx) and DVE are the
    leading costs.
  - DVE builds bf16 one-hot masks via tensor_scalar is_equal (2x rate) and
    multiplies; the d-reduction is split 2/3 DVE reduce_sum / 1/3 ACT
    accumulate; ACT applies sigmoid; the host casts and unpermutes.
"""

import numpy as np
import ml_dtypes

BF16 = ml_dtypes.bfloat16

N_DRUG, N_DIS, D = 8000, 18000, 128
N_REL_DIR, E = 3, 200000
N_CORES = 8
EPC = E // N_CORES          # 25000 edges per core per relation
T2 = 200                    # chunks per (relation, core); 25600 edge slots
EL = T2 * 128

_cache = {}
_last = {}


def _blk_of(t, nb):
    return t * nb // T2


def _build_nc(cfg):
    import concourse.bacc as bacc
    import concourse.mybir as mybir
    from concourse.tile import TileContext

    f32 = mybir.dt.float32
    bf16 = mybir.dt.bfloat16
    i16 = mybir.dt.int16
    u8 = mybir.dt.uint8

    nblk_f, nblk_r, _t2 = cfg
    assert _t2 == T2
    nblk = {0: nblk_f, 1: nblk_r}

    nc = bacc.Bacc("TRN2", target_bir_lowering=False, debug=False,
                   num_devices=N_CORES, num_swdge_queues=4)

    t_hd = nc.dram_tensor("hd", (N_DRUG, D), bf16, kind="ExternalInput")
    t_hs = nc.dram_tensor("hs", (N_DIS, D), bf16, kind="ExternalInput")
    t_u = [nc.dram_tensor(f"u{r}", (nblk[r >= 3] * 128, D), bf16,
                          kind="ExternalInput") for r in range(6)]
    t_iota = nc.dram_tensor("iota", (128, 2), f32, kind="ExternalInput")
    t_ids = [nc.dram_tensor(f"ids{r}", (128, EL), u8,
                            kind="ExternalInput") for r in range(6)]
    t_iv = [nc.dram_tensor(f"iv{r}", (128, EL // 16), i16,
                           kind="ExternalInput") for r in range(6)]
    t_out = [nc.dram_tensor(f"scores{r}", (128, T2), f32,
                            kind="ExternalOutput") for r in range(6)]

    with TileContext(nc) as tc:
        with tc.tile_pool(name="cst", bufs=1) as cst, \
             tc.tile_pool(name="mp", bufs=2) as mp, \
             tc.tile_pool(name="gp", bufs=3) as gp, \
             tc.tile_pool(name="gvp", bufs=5) as gvp, \
             tc.tile_pool(name="pp", bufs=4, space="PSUM") as pp:
            iota = cst.tile([128, 2], f32)
            nc.sync.dma_start(iota[:], t_iota[:])
            for r in range(6):
                dr = int(r >= 3)
                NB = nblk[dr]
                v_tab = t_hs if dr == 0 else t_hd

                # u range -> SBUF (row 128b+p at [p, b, :]); W-prescaled
                # bf16 on the host
                u_sb = mp.tile([128, NB, D], bf16, tag=f"usb{dr}")
                nc.sync.dma_start(
                    u_sb[:], t_u[r][:].rearrange("(b p) d -> p b d", p=128))

                iv = mp.tile([128, EL // 16], i16, tag="iv")
                nc.sync.dma_start(iv[:], t_iv[r][:])
                scores = mp.tile([128, T2], f32, tag="scores")

                batches = [40] * (T2 // 40) + ([T2 % 40] if T2 % 40 else [])
                c0 = 0
                for b, kbn in enumerate(batches):
                    nb_i = kbn * 128
                    gv = gvp.tile([128, 40, D], bf16, tag="gv")
                    # split each batch across the 4 SWDGE queues: desc-gen for
                    # queue q runs on Q7 core pair q, so the four quarters
                    # generate concurrently
                    # the queue that also carries this batch's u-gather gets
                    # a smaller v share so per-pair desc-gen is balanced
                    # fine-grained, pair-balanced issue: pairs 0/1 take
                    # 2x7 v-chunks, pairs 2/3 take 6 v-chunks (they also
                    # carry the 8-chunk u-gathers) -> 14 chunks per pair
                    gx = 0
                    qn = max(1, -(-kbn // 4))
                    segs = []
                    left, q = kbn, 0
                    while left > 0:
                        take = min(qn, left)
                        segs.append((q % 4, take))
                        left -= take
                        q += 1
                    k0 = 0
                    for q, sz in segs:
                        k1 = k0 + sz
                        nc.gpsimd.dma_gather(
                            gv[:, k0:k1, :], v_tab[:],
                            iv[:, (c0 + k0) * 8:(c0 + k1) * 8],
                            sz * 128, sz * 128, D,
                            elem_step=D, single_packet=False, queue_num=q)
                        k0 = k1
                    noh = kbn
                    ids = gp.tile([128, 40 * 128], u8, tag="ids")
                    nc.sync.dma_start(
                        ids[:, :noh * 128],
                        t_ids[r][:, c0 * 128:(c0 + kbn) * 128])
                    oh_lo = gp.tile([128, 40 * 128], bf16, tag="ohlo")
                    nc.vector.tensor_scalar(
                        oh_lo[:, :noh * 128], ids[:, :noh * 128],
                        iota[:, 1:2], None, op0=mybir.AluOpType.is_equal)
                    for g0 in range(0, kbn, 4):
                        gn = min(4, kbn - g0)
                        if g0 + gn <= gx:
                            usrc = gu[:, g0:g0 + gn, :]
                        elif g0 >= gx:
                            ps = pp.tile([128, 4, D], f32, tag="ps")
                            for i in range(g0, g0 + gn):
                                t = c0 + i
                                blk = _blk_of(t, NB)
                                j = i - gx
                                nc.tensor.matmul(
                                    ps[:, i - g0, :],
                                    lhsT=oh_lo[:, j * 128:(j + 1) * 128],
                                    rhs=u_sb[:, blk, :],
                                    start=True, stop=True)
                            usrc = ps[:, :gn, :]
                        else:
                            raise AssertionError("gx must be multiple of 4")
                        prod = gp.tile([128, 4, D], f32, tag="prod")
                        nc.vector.tensor_tensor(
                            prod[:, :gn, :].rearrange("p a b -> p (a b)"),
                            usrc.rearrange("p a b -> p (a b)"),
                            gv[:, g0:g0 + gn, :].rearrange("p a b -> p (a b)"),
                            op=mybir.AluOpType.mult)
                        # reduction split between scalar engine (4x slower
                        # per chunk but otherwise idle) and DVE
                        if (g0 // 4) % 3 != 0:
                            nc.vector.reduce_sum(
                                scores[:, c0 + g0:c0 + g0 + gn],
                                prod[:, :gn, :], axis=mybir.AxisListType.X)
                        else:
                            acts = cst.tile([128, D], f32, tag="actout")
                            for i in range(gn):
                                nc.scalar.activation(
                                    acts[:], prod[:, i, :],
                                    mybir.ActivationFunctionType.Copy,
                                    accum_out=scores[:, c0 + g0 + i:c0 + g0 + i + 1])
                    c0 += kbn

                sig = mp.tile([128, T2], f32, tag="sig")
                nc.scalar.activation(
                    sig[:], scores[:], mybir.ActivationFunctionType.Sigmoid)
                nc.sync.dma_start(t_out[r][:], sig[:])

    nc.compile()
    return nc


def _wrap_idx(idx):
    n = idx.shape[0]
    w = idx.reshape(n // 16, 16).T.astype(np.int16)
    return np.ascontiguousarray(np.tile(w, (8, 1)))


def _pack_schedule(u_local, v_idx, nblk):
    """Pack edges (sorted by u_local) into T2 chunks of 128 where chunk t may
    only use rows assigned to block blk_t = t*nblk//T2, at most 128 distinct
    rows per block.  Returns (ids, v16, edge_of_slot, vpos) or None."""
    rows, starts, counts = np.unique(u_local, return_index=True,
                                     return_counts=True)
    nrows = rows.shape[0]
    ids = np.zeros(EL, np.uint8)
    v16 = np.zeros(EL, np.int16)
    eos = np.full(EL, -1, np.int64)
    vpos = np.full(int(u_local[-1]) + 1, -1, np.int64)

    blk_of_t = np.array([_blk_of(t, nblk) for t in range(T2)], np.int64)
    t_first = np.searchsorted(blk_of_t, np.arange(nblk), side="left")
    t_last = np.searchsorted(blk_of_t, np.arange(nblk), side="right")
    ri = 0
    for b in range(nblk):
        cap = 128 * int(t_last[b] - t_first[b])
        slot0 = 128 * int(t_first[b])
        used = 0
        rib = 0
        while ri < nrows and rib < 128 and used + int(counts[ri]) <= cap:
            c = int(counts[ri])
            sx = int(starts[ri])
            sl = slot0 + used
            ids[sl:sl + c] = rib
            v16[sl:sl + c] = v_idx[sx:sx + c].astype(np.int16)
            eos[sl:sl + c] = np.arange(sx, sx + c)
            vpos[int(rows[ri])] = 128 * b + rib
            used += c
            rib += 1
            ri += 1
    if ri != nrows:
        return None
    return ids, v16, eos, vpos


def _prepare(rels, sliced, nblk_f, nblk_r, W, iota):
    slot_maps = [[None] * N_CORES for _ in range(6)]
    in_maps = []
    hd16 = rels[0][2].astype(BF16)
    hs16 = rels[3][2].astype(BF16)
    for c in range(N_CORES):
        m = {"hd": hd16, "hs": hs16, "iota": iota}
        for r in range(6):
            dr = int(r >= 3)
            nblk = nblk_f if dr == 0 else nblk_r
            u_local, v_idx, lo = sliced[r][c]
            packed = _pack_schedule(u_local, v_idx, nblk)
            if packed is None:
                return None, None, (r, c)
            ids, v16, eos, vpos = packed
            tab = rels[r][2]
            span = vpos.shape[0]
            urows = np.zeros((nblk * 128, D), np.float32)
            valid = vpos >= 0
            urows[vpos[valid]] = (tab[lo:lo + span][valid]
                                  * W[r][None, :]).astype(np.float32)
            m[f"u{r}"] = urows.astype(BF16)
            m[f"ids{r}"] = np.ascontiguousarray(
                np.broadcast_to(ids[None, :], (128, EL)))
            m[f"iv{r}"] = _wrap_idx(v16)
            slot_maps[r][c] = eos
        in_maps.append(m)
    return slot_maps, in_maps, None


def kernel(h_drug, h_disease, W, drug_src, dis_dst, dis_src, drug_dst):
    from concourse.bass_utils import run_bass_kernel_spmd

    h_drug = np.asarray(h_drug, dtype=np.float32)
    h_disease = np.asarray(h_disease, dtype=np.float32)
    W = np.asarray(W, dtype=np.float32)

    rels = []
    for r in range(3):
        rels.append((np.asarray(drug_src[r]), np.asarray(dis_dst[r]), h_drug))
    for r in range(3):
        rels.append((np.asarray(dis_src[r]), np.asarray(drug_dst[r]), h_disease))

    perms = []
    sliced = []
    for r in range(6):
        u_idx, v_idx, _ = rels[r]
        perm = np.argsort(u_idx, kind="stable")
        perms.append(perm)
        us, vs = u_idx[perm], v_idx[perm]
        sl = []
        for c in range(N_CORES):
            ui = us[c * EPC:(c + 1) * EPC]
            vi = vs[c * EPC:(c + 1) * EPC]
            lo = int(ui[0])
            sl.append((ui - lo, vi, lo))
        sliced.append(sl)

    def span_max(dr):
        sp = 0
        for r in (range(3) if dr == 0 else range(3, 6)):
            for c in range(N_CORES):
                sp = max(sp, int(sliced[r][c][0][-1]) + 1)
        return sp

    nblk_f = max(2, -(-span_max(0) // 112))
    nblk_r = max(2, -(-span_max(1) // 112))

    iota = np.empty((128, 2), np.float32)
    iota[:, 0] = np.arange(128)
    iota[:, 1] = np.arange(128)

    slot_maps = in_maps = None
    for _attempt in range(6):
        slot_maps, in_maps, fail = _prepare(rels, sliced, nblk_f, nblk_r,
                                            W, iota)
        if fail is None:
            break
        if fail[0] < 3:
            nblk_f += 1
        else:
            nblk_r += 1
    else:
        raise RuntimeError("could not build a feasible chunk schedule")

    cfg = (nblk_f, nblk_r, T2)
    if cfg not in _cache:
        _cache[cfg] = _build_nc(cfg)
    nc = _cache[cfg]

    res = run_bass_kernel_spmd(nc, in_maps, core_ids=list(range(N_CORES)))
    _last["exec_time_ns"] = res.exec_time_ns
    if res.instructions_and_trace is not None:
        _last["trace_path"] = res.instructions_and_trace[1]

    out = np.empty((6, E), np.float32)
    for r in range(6):
        sorted_scores = np.empty(EPC * N_CORES, np.float32)
        for c in range(N_CORES):
            s = res.results[c][f"scores{r}"]       # [128, T2]
            flat = s.T.reshape(-1)                 # slot j = t*128+p
            eos = slot_maps[r][c]
            valid = eos >= 0
            sorted_scores[c * EPC + eos[valid]] = flat[valid]
        out[r, perms[r]] = sorted_scores
    return out

